# revision 1
# baseline (speedup 1.0000x reference)
"""GATv2Conv layer on 8 Trainium2 NeuronCores (Bass/Tile).

Strategy (edge-parallel, dst-sorted, zero cross-core collectives):
  - Host sorts edges by dst and partitions nodes into 8 contiguous ranges of
    6250; each core owns all edges targeting its node range (~100k edges).
  - Within a core, dst nodes are tiled 128 at a time (49 tiles); each tile's
    edges are padded to 18 chunks of 128 edges (max real count is 2174).
  - Per 128-edge chunk, everything is PE matmuls in feature-major layout:
      s^T[f,e]   = We^T @ eaT  +  xr^T @ Onehot[n,e]  +  (xl[src])^T (via I)
      logits^T   = lrelu(s)^T-slice as lhsT  @  att_blockdiag   -> [e, 4]
      scatter    = Onehot[e,n]^T @ (ex * xl_g | ex)  accumulated in PSUM
  - Softmax skips the segment-max (|logit| <= ~8 for this data, exp is safe
    in fp32); denominator is scattered alongside the messages, and the
    division happens once per node.
  - The xl table (x @ Wl, no bias) is built redundantly by every core and
    gathered from its own HBM with int32 row indices via indirect DMA.
    bl is algebraically moved: s gets it via the lrelu bias (per-feature,
    partition axis), and the aggregation gets it post-division (bl is zero
    in practice so that add is skipped).
"""

import sys

import numpy as np

sys.path.insert(0, "/opt/trn_rl_repo")

N, E, D, H, C, EDGE_DIM = 50000, 800000, 128, 4, 32, 16
NEG_SLOPE = 0.2
LN_EPS = 1e-5
N_CORES = 8
NPC = N // N_CORES            # 6250 nodes per core
TILES = (NPC + 127) // 128    # 49 dst tiles per core
NPAD = TILES * 128            # 6272
CHUNKS = 18                   # 128-edge chunks per tile (max observed 17)
TE = CHUNKS * 128             # 2304 padded edges per tile
NT = ((N // 128) + 1) * 128   # xl table rows padded: 50048
GROUPS = [(0, 4), (4, 8), (8, 12), (12, 16), (16, 18)]
CH_A = 9                      # chunks gathered from table half A (src < HALF)
NIA = CH_A * 128              # 1152 indices per half-gather
ICOLS = NIA // 16             # 72 int16 index columns per half
HALF = 25088                  # xl-table split row (multiple of 128)

TRACE = False                 # set by test.py to capture a HW profile
import os as _os
def _envint(n):
    v = _os.environ.get(n)
    return None if v is None else int(v)
BISECT_TILES = _envint("BISECT_TILES")
BISECT_SLABS = _envint("BISECT_SLABS")
SKIP_GATHER = _os.environ.get("SKIP_GATHER") == "1"
GATHER_EXT = _os.environ.get("GATHER_EXT") == "1"
LAST_EXEC_TIME_NS = None
LAST_RESULTS = None

_CACHE = {}


def _np_dt(mdt):
    from concourse import mybir
    return np.dtype(mybir.dt.np(mdt))




def _emit_gathers(nc, bass, table_sb, xlgT_sb, idx_sb):
    """Two SBUF-source transposed gathers: xlgT_sb[:, h*NIA + e] = xl[src_e]^T."""
    for h in range(2):
        base = xlgT_sb[:, h * NIA : (h + 1) * NIA]
        out3 = bass.AP(
            base.tensor, base.offset, [list(base.ap[0]), [NIA, 1], [1, NIA]]
        )
        src_ap = (
            table_sb[:, 0 : HALF // 128 * 128]
            if h == 0
            else table_sb[:, HALF // 128 * 128 : NT // 128 * 128]
        )
        nc.gpsimd.dma_gather(
            out_ap=out3,
            in_ap=src_ap,
            idxs_ap=idx_sb[:, h * ICOLS : (h + 1) * ICOLS],
            num_idxs=NIA,
            num_idxs_reg=NIA,
            elem_size=128,
            transpose=True,
            single_packet=False,
            sbuf_tokens_per_rank=128,
            sbuf_free_dim_per_rank=256,
            sbuf_byte_offset=0,
        )

def _build_nc(flags):
    """flags = (add_post, use_gamma, use_beta, half) — half is the xl-table
    split row for the int16-indexed two-half dma_gather."""
    import concourse.bacc as bacc
    import concourse.bass as bass
    import concourse.tile as tile
    from concourse import mybir

    add_post, use_gamma, use_beta, half = flags
    f32, bf16, i32 = mybir.dt.float32, mybir.dt.bfloat16, mybir.dt.int32
    fp8 = mybir.dt.float8e4
    i16 = mybir.dt.int16
    AF = mybir.ActivationFunctionType
    OP = mybir.AluOpType

    nc = bacc.Bacc(None, target_bir_lowering=False)

    # --- shared (same array for all cores) inputs -------------------------
    xT_d = nc.dram_tensor("xT", [128, NT], bf16, kind="ExternalInput")
    wl_d = nc.dram_tensor("wl", [128, 128], bf16, kind="ExternalInput")
    wr_d = nc.dram_tensor("wr", [128, 128], bf16, kind="ExternalInput")
    we_d = nc.dram_tensor("we", [16, 128], bf16, kind="ExternalInput")
    attb_d = nc.dram_tensor("attb", [128, 4], bf16, kind="ExternalInput")
    ident_d = nc.dram_tensor("ident", [128, 128], bf16, kind="ExternalInput")
    lrb_d = nc.dram_tensor("lrb", [128, 1], f32, kind="ExternalInput")
    if add_post:
        post_d = nc.dram_tensor("post", [128, 128], f32, kind="ExternalInput")
    if use_gamma:
        gam_d = nc.dram_tensor("gam", [128, 128], f32, kind="ExternalInput")
    if use_beta:
        bet_d = nc.dram_tensor("bet", [128, 128], f32, kind="ExternalInput")

    # --- per-core inputs ---------------------------------------------------
    idx_d = nc.dram_tensor("idx", [TILES * 128, 2 * ICOLS], i16, kind="ExternalInput")
    oh_d = nc.dram_tensor("oh", [TILES * 128, 2 * TE], fp8, kind="ExternalInput")
    eaT_d = nc.dram_tensor("eaT", [TILES * 16, TE], bf16, kind="ExternalInput")
    xoT_d = nc.dram_tensor("xoT", [128, NPAD], bf16, kind="ExternalInput")
    xown_d = nc.dram_tensor("xown", [NPAD, 128], f32, kind="ExternalInput")
    if GATHER_EXT:
        xlext_d = nc.dram_tensor("xlext", [NT, 128], bf16, kind="ExternalInput")
    out_d = nc.dram_tensor("out", [NPAD, 128], f32, kind="ExternalOutput")

    with tile.TileContext(nc) as tc:
        with (
            tc.tile_pool(name="tabp", bufs=1) as tabp,
            tc.tile_pool(name="constp", bufs=1) as constp,
        ):
            table_sb = tabp.tile([128, NT], bf16)

            wl_sb = constp.tile([128, 128], bf16)
            nc.sync.dma_start(wl_sb, wl_d[:])
            wr_sb = constp.tile([128, 128], bf16)
            nc.sync.dma_start(wr_sb, wr_d[:])
            we_sb = constp.tile([16, 128], bf16)
            nc.sync.dma_start(we_sb, we_d[:])
            attb_sb = constp.tile([128, 4], bf16)
            nc.sync.dma_start(attb_sb, attb_d[:])
            ident_sb = constp.tile([128, 128], bf16)
            nc.sync.dma_start(ident_sb, ident_d[:])
            lrb_sb = constp.tile([128, 1], f32)
            nc.sync.dma_start(lrb_sb, lrb_d[:])
            zero_sb = constp.tile([128, 1], f32)
            nc.vector.memset(zero_sb, 0.0)
            eps_sb = constp.tile([128, 1], f32)
            nc.vector.memset(eps_sb, LN_EPS)
            if add_post:
                post_sb = constp.tile([128, 128], f32)
                nc.sync.dma_start(post_sb, post_d[:])
            if use_gamma:
                gam_sb = constp.tile([128, 128], f32)
                nc.sync.dma_start(gam_sb, gam_d[:])
            if use_beta:
                bet_sb = constp.tile([128, 128], f32)
                nc.sync.dma_start(bet_sb, bet_d[:])

            # ---------------- phase A: xl table = x @ Wl (bf16) ----------
            with (
                tc.tile_pool(name="xslabp", bufs=3) as xslabp,
                tc.tile_pool(name="psAp", bufs=4, space="PSUM") as psAp,
            ):
                SLAB = 2048
                _slabs = list(range(0, NT, SLAB))
                if BISECT_SLABS is not None:
                    _slabs = _slabs[:BISECT_SLABS]
                for s in _slabs:
                    w = min(SLAB, NT - s)
                    xslab = xslabp.tile([128, SLAB], bf16)
                    nc.sync.dma_start(xslab[:, :w], xT_d[:, s : s + w])
                    for k in range(w // 128):
                        psA = psAp.tile([128, 128], f32, space="PSUM")
                        nc.tensor.matmul(
                            out=psA,
                            lhsT=xslab[:, k * 128 : (k + 1) * 128],
                            rhs=wl_sb,
                            start=True,
                            stop=True,
                        )
                        nc.any.tensor_copy(
                            table_sb[:, s + k * 128 : s + (k + 1) * 128], psA
                        )

            # ---------------- phase B: edge tiles ------------------------
            with (
                tc.tile_pool(name="idxp", bufs=3) as idxp,
                tc.tile_pool(name="ohp", bufs=3) as ohp,
                tc.tile_pool(name="eatp", bufs=3) as eatp,
                tc.tile_pool(name="xotp", bufs=2) as xotp,
                tc.tile_pool(name="xlgp", bufs=2) as xlgp,
                tc.tile_pool(name="xrp", bufs=2) as xrp,
                tc.tile_pool(name="lrsp", bufs=2) as lrsp,
                tc.tile_pool(name="exp_", bufs=2) as exp_,
                tc.tile_pool(name="msgp", bufs=2) as msgp,
                tc.tile_pool(name="nodep", bufs=2) as nodep,
                tc.tile_pool(name="psSp", bufs=2, space="PSUM") as psSp,
                tc.tile_pool(name="psLp", bufs=1, space="PSUM") as psLp,
                tc.tile_pool(name="psOp", bufs=2, space="PSUM") as psOp,
                tc.tile_pool(name="psXp", bufs=1, space="PSUM") as psXp,
                tc.tile_pool(name="psGp", bufs=2, space="PSUM") as psGp,
            ):
                xoT_all = xotp.tile([128, NPAD], bf16, bufs=1)
                nc.sync.dma_start(xoT_all, xoT_d[:, :])
                _tiles = range(TILES) if BISECT_TILES is None else range(BISECT_TILES)
                for t in _tiles:
                    idx_sb = idxp.tile([128, 2 * ICOLS], i16)
                    nc.sync.dma_start(idx_sb, idx_d[t * 128 : (t + 1) * 128, :])
                    oh_sb = ohp.tile([128, 2 * TE], fp8)
                    nc.sync.dma_start(oh_sb, oh_d[t * 128 : (t + 1) * 128, :])
                    eaT_sb = eatp.tile([16, TE], bf16)
                    nc.sync.dma_start(eaT_sb, eaT_d[t * 16 : (t + 1) * 16, :])

                    # xr tile for own 128 nodes (no bias; folded into lrelu)
                    psX = psXp.tile([128, 128], f32, space="PSUM")
                    nc.tensor.matmul(
                        out=psX,
                        lhsT=xoT_all[:, t * 128 : (t + 1) * 128],
                        rhs=wr_sb,
                        start=True,
                        stop=True,
                    )
                    xr_sb = xrp.tile([128, 128], bf16)
                    nc.any.tensor_copy(xr_sb, psX)

                    # gather xl columns (feature-major) for this tile's edges
                    xlgT_sb = xlgp.tile([128, TE], bf16)
                    if SKIP_GATHER:
                        nc.vector.memset(xlgT_sb[:, :], 0.0)
                    else:
                        _emit_gathers(nc, bass, table_sb, xlgT_sb, idx_sb)


                    # s^T accumulation + lrelu, per chunk-group
                    lrsT_sb = lrsp.tile([128, TE], bf16)
                    ex_sb = exp_.tile([128, CHUNKS, 4], bf16)
                    msg_sb = msgp.tile([128, CHUNKS, 132], bf16)
                    for g0, g1 in GROUPS:
                        ng = g1 - g0
                        psS = psSp.tile([128, 512], f32, space="PSUM", tag="psS")
                        for i, ch in enumerate(range(g0, g1)):
                            cs = slice(i * 128, (i + 1) * 128)
                            ecs = slice(ch * 128, (ch + 1) * 128)
                            nc.tensor.matmul(
                                out=psS[:, cs], lhsT=we_sb, rhs=eaT_sb[:, ecs],
                                start=True, stop=False,
                            )
                            nc.tensor.matmul(
                                out=psS[:, cs], lhsT=xr_sb,
                                rhs=oh_sb[:, TE + ch * 128 : TE + (ch + 1) * 128],
                                start=False, stop=False,
                            )
                            nc.tensor.matmul(
                                out=psS[:, cs], lhsT=ident_sb, rhs=xlgT_sb[:, ecs],
                                start=False, stop=True,
                            )
                        nc.scalar.activation(
                            lrsT_sb[:, g0 * 128 : g1 * 128],
                            psS[:, : ng * 128],
                            AF.Prelu,
                            bias=lrb_sb[:, :],
                            scale=1.0,
                            alpha=NEG_SLOPE,
                        )

                        # logits (edge-major) then exp
                        psL = psLp.tile([128, 4, 4], f32, space="PSUM", tag="psL")
                        for i, ch in enumerate(range(g0, g1)):
                            nc.tensor.matmul(
                                out=psL[:, i, :],
                                lhsT=lrsT_sb[:, ch * 128 : (ch + 1) * 128],
                                rhs=attb_sb,
                                start=True,
                                stop=True,
                            )
                        nc.scalar.activation(
                            ex_sb[:, g0:g1, :], psL[:, :ng, :], AF.Exp,
                            bias=zero_sb[:, :],
                        )

                        # transpose gathered columns back to edge-major in PSUM
                        psG = psGp.tile([128, 512], f32, space="PSUM", tag="psG")
                        for i, ch in enumerate(range(g0, g1)):
                            nc.tensor.matmul(
                                out=psG[:, i * 128 : (i + 1) * 128],
                                lhsT=xlgT_sb[:, ch * 128 : (ch + 1) * 128],
                                rhs=ident_sb,
                                start=True,
                                stop=True,
                            )
                        # messages: msg[:, ch, :128] = xlg*ex ; msg[:, ch, 128:] = ex
                        exb = ex_sb[:, g0:g1, :]
                        ex_bcast = exb.broadcast_to([128, ng, 4, 32])
                        psG3 = bass.AP(
                            psG.tensor, psG.offset,
                            [list(psG.ap[0]), [128, ng], [1, 128]],
                        )
                        nc.vector.tensor_tensor(
                            out=msg_sb[:, g0:g1, 0:128],
                            in0=psG3,
                            in1=ex_bcast,
                            op=OP.mult,
                        )
                        nc.vector.tensor_copy(msg_sb[:, g0:g1, 128:132], exb)

                    # scatter-sum into [node, 128 msg + 4 denom]
                    psO = psOp.tile([128, 132], f32, space="PSUM", tag="psO")
                    for ch in range(CHUNKS):
                        nc.tensor.matmul(
                            out=psO,
                            lhsT=oh_sb[:, ch * 128 : (ch + 1) * 128],
                            rhs=msg_sb[:, ch, :],
                            start=(ch == 0),
                            stop=(ch == CHUNKS - 1),
                        )

                    # ---- node phase: normalize, gelu, residual, layernorm
                    den_sb = nodep.tile([128, 4], f32, tag="den")
                    nc.vector.tensor_scalar_add(den_sb, psO[:, 128:132], 1e-16)
                    rden_sb = nodep.tile([128, 4], f32, tag="rden")
                    nc.vector.reciprocal(rden_sb, den_sb)
                    rden_bc = rden_sb.broadcast_to([128, 4, 32])
                    h_sb = nodep.tile([128, 128], f32, tag="h")
                    nc.vector.tensor_tensor(
                        out=h_sb, in0=psO[:, 0:128], in1=rden_bc, op=OP.mult
                    )
                    if add_post:
                        nc.vector.tensor_add(h_sb, h_sb, post_sb)
                    g_sb = nodep.tile([128, 128], f32, tag="g")
                    nc.scalar.activation(g_sb, h_sb, AF.Gelu, bias=zero_sb[:, :])
                    xo_sb = nodep.tile([128, 128], f32, tag="xo")
                    nc.sync.dma_start(xo_sb, xown_d[t * 128 : (t + 1) * 128, :])
                    r_sb = nodep.tile([128, 128], f32, tag="r")
                    musum = nodep.tile([128, 1], f32, tag="musum")
                    nc.vector.scalar_tensor_tensor(
                        out=r_sb, in0=g_sb, scalar=1.0, in1=xo_sb,
                        op0=OP.mult, op1=OP.add, accum_out=musum,
                    )
                    mu = nodep.tile([128, 1], f32, tag="mu")
                    nc.vector.tensor_scalar_mul(mu, musum, 1.0 / 128.0)
                    c_sb = nodep.tile([128, 128], f32, tag="c")
                    nc.vector.tensor_scalar(
                        out=c_sb, in0=r_sb, scalar1=mu, scalar2=None, op0=OP.subtract
                    )
                    c2_sb = nodep.tile([128, 128], f32, tag="c2")
                    varsum = nodep.tile([128, 1], f32, tag="varsum")
                    nc.vector.scalar_tensor_tensor(
                        out=c2_sb, in0=c_sb, scalar=1.0, in1=c_sb,
                        op0=OP.mult, op1=OP.mult, accum_out=varsum,
                    )
                    std = nodep.tile([128, 1], f32, tag="std")
                    nc.scalar.activation(
                        std, varsum, AF.Sqrt, bias=eps_sb[:, :], scale=1.0 / 128.0
                    )
                    rstd = nodep.tile([128, 1], f32, tag="rstd")
                    nc.vector.reciprocal(rstd, std)
                    o_sb = nodep.tile([128, 128], f32, tag="o")
                    nc.vector.tensor_scalar(
                        out=o_sb, in0=c_sb, scalar1=rstd, scalar2=None, op0=OP.mult
                    )
                    if use_gamma:
                        nc.vector.tensor_mul(o_sb, o_sb, gam_sb)
                    if use_beta:
                        nc.vector.tensor_add(o_sb, o_sb, bet_sb)
                    nc.sync.dma_start(out_d[t * 128 : (t + 1) * 128, :], o_sb)

    return nc


def _host_prep(x, edge_index, edge_attr, Wl, bl, Wr, br, We, att, bias, gamma, beta):
    import ml_dtypes

    bf16 = ml_dtypes.bfloat16
    fp8 = ml_dtypes.float8_e4m3

    src = np.ascontiguousarray(edge_index[0]).astype(np.int64)
    dst = np.ascontiguousarray(edge_index[1]).astype(np.int64)
    order = np.argsort(dst, kind="stable")
    ssrc = src[order].astype(np.int32)
    sdst = dst[order].astype(np.int32)
    sea = np.ascontiguousarray(edge_attr, dtype=np.float32)[order]

    xTp = np.zeros((128, NT), dtype=bf16)
    xTp[:, :N] = x.T.astype(bf16)

    shared = {
        "xT": xTp,
        "wl": Wl.astype(bf16),
        "wr": Wr.astype(bf16),
        "we": We.astype(bf16),
        "ident": np.eye(128, dtype=bf16),
        "lrb": (bl + br).astype(np.float32).reshape(128, 1),
    }
    attb = np.zeros((128, 4), dtype=np.float32)
    for h in range(H):
        attb[h * C : (h + 1) * C, h] = att[h]
    shared["attb"] = attb.astype(bf16)

    post = bl + bias  # added after the denom division, before gelu
    add_post = bool(np.any(post != 0.0))
    use_gamma = bool(np.any(gamma != 1.0))
    use_beta = bool(np.any(beta != 0.0))

    half = HALF  # stripe-aligned table split for the SBUF-source gathers
    if add_post:
        shared["post"] = np.broadcast_to(post.astype(np.float32), (128, 128)).copy()
    if use_gamma:
        shared["gam"] = np.broadcast_to(gamma.astype(np.float32), (128, 128)).copy()
    if use_beta:
        shared["bet"] = np.broadcast_to(beta.astype(np.float32), (128, 128)).copy()

    in_maps = []
    for c in range(N_CORES):
        n0 = c * NPC
        e0 = np.searchsorted(sdst, n0)
        e1 = np.searchsorted(sdst, n0 + NPC)
        csrc = ssrc[e0:e1]
        cdst = sdst[e0:e1] - n0
        cea = sea[e0:e1]

        idx = np.zeros((TILES, 128, 2 * ICOLS), dtype=np.int16)
        eaT = np.zeros((TILES, 16, TE), dtype=bf16)
        oh = np.zeros((TILES, 128, 2, CHUNKS, 128), dtype=fp8)  # [t,p,(en|ne),ch,n]
        ohen = oh[:, :, 0].reshape(TILES, 128, CHUNKS, 128)
        ohne = oh[:, :, 1].reshape(TILES, 128, TE)
        tb = np.searchsorted(cdst, np.arange(0, NPAD + 1, 128))
        one8 = np.ones((), dtype=fp8)
        prow = np.arange(128) % 16
        for t in range(TILES):
            sl = slice(tb[t], tb[t + 1])
            es = csrc[sl]
            ed = (cdst[sl] - t * 128).astype(np.int64)
            ea_t = cea[sl]
            # split by src table half; each half padded to CH_A chunks
            mA = es < half
            for hi, (msk, base) in enumerate(((mA, 0), (~mA, half))):
                k = int(msk.sum())
                assert k <= NIA, f"core {c} tile {t} half {hi}: {k} > {NIA}"
                j = np.arange(k) + hi * NIA  # slot within the tile
                idlist = np.zeros(NIA, np.int16)
                idlist[:k] = (es[msk] - base).astype(np.int16)
                idx[t, :, hi * ICOLS : (hi + 1) * ICOLS] = idlist.reshape(
                    ICOLS, 16
                ).T[prow]
                eaT[t, :, j] = ea_t[msk].astype(bf16)
                ohen[t, j % 128, j // 128, ed[msk]] = one8
                ohne[t, ed[msk], j] = one8

        xoT = np.zeros((128, NPAD), dtype=bf16)
        xoT[:, :NPC] = x[n0 : n0 + NPC].T.astype(bf16)
        xown = np.zeros((NPAD, 128), dtype=np.float32)
        xown[:NPC] = x[n0 : n0 + NPC]

        m = dict(shared)
        m.update(
            idx=idx.reshape(TILES * 128, 2 * ICOLS),
            eaT=eaT.reshape(TILES * 16, TE),
            oh=oh.reshape(TILES * 128, 2 * TE),
            xoT=xoT,
            xown=xown,
        )
        in_maps.append(m)
    return in_maps, (add_post, use_gamma, use_beta, half)


def kernel(x, edge_index, edge_attr, Wl, bl, Wr, br, We, att, bias, gamma, beta):
    global LAST_EXEC_TIME_NS, LAST_RESULTS
    x = np.asarray(x, np.float32)
    edge_index = np.asarray(edge_index)
    edge_attr = np.asarray(edge_attr, np.float32)
    Wl = np.asarray(Wl, np.float32)
    bl = np.asarray(bl, np.float32)
    Wr = np.asarray(Wr, np.float32)
    br = np.asarray(br, np.float32)
    We = np.asarray(We, np.float32)
    att = np.asarray(att, np.float32)
    bias = np.asarray(bias, np.float32)
    gamma = np.asarray(gamma, np.float32)
    beta = np.asarray(beta, np.float32)

    in_maps, flags = _host_prep(
        x, edge_index, edge_attr, Wl, bl, Wr, br, We, att, bias, gamma, beta
    )

    if flags not in _CACHE:
        nc = _build_nc(flags)
        nc.finalize()
        _CACHE[flags] = nc
    nc = _CACHE[flags]

    from concourse.bass_utils import run_bass_kernel_spmd

    res = run_bass_kernel_spmd(
        nc, in_maps, list(range(N_CORES)), trace=TRACE
    )
    LAST_RESULTS = res
    LAST_EXEC_TIME_NS = res.exec_time_ns
    out = np.concatenate([res.results[c]["out"][:NPC] for c in range(N_CORES)], axis=0)
    return out.astype(np.float32)

